# revision 24
# baseline (speedup 1.0000x reference)
"""AttentionBlock (InstanceNorm + single-head self-attention over 64x64 pixels
+ residual) on 8 Trainium2 NeuronCores.

Sharding: core = (batch b = core//2, query-half h = core%2). Each core gets the
full 512x4096 plane of its batch (columns rolled so its 2048 query pixels are
columns 0..2047), computes norm + K/V for all 4096 pixels and Q for its 2048,
runs softmax(Q^T K / sqrt(C)) V and the output projection for its half, and
returns a [512, 2048] shard. No collectives; the graph is SPMD-identical and
per-core differences enter only through the input data.

The InstanceNorm is folded into the projections: q/k/v = W @ ((x-mu)*rstd) + b
= (W*rstd) @ x + (b + W @ nmb) with nmb = -mu*rstd, so the projection matmuls
run directly on raw x as soon as the per-channel stats are known - the
normalized tensor is never materialized. The bias corrections W @ nmb are
[C,1] matmuls (cheap).

The attention is computed in the transposed orientation,
logitsT[j, i] = <k_j, q_i>, so the softmax denominator sum_j exp() is a
ones-matmul over the partition axis and exp() needs no max-subtraction
(logits are bounded for this data distribution; exp carries a -5 offset so
fp8 storage cannot overflow - the offset cancels exactly in U/Z).
The v bias passes through softmax exactly (attention rows sum to 1), so it is
folded into the residual on the host: xq' = xq + wp @ bv + bp. The v "const"
term wv @ nmb also passes through softmax; it is projected through wp on-chip
([C,1] matmuls) and added in the output epilogue.

QK^T and exp()V run as fp8(e4m3) DoubleRow matmuls - 256-deep contraction per
instruction, half the PE instructions of bf16 - with fp32 PSUM accumulation.
Everything else is bf16 with fp32 accumulation.
"""

import numpy as np
import ml_dtypes

import concourse.bass as bass
import concourse.mybir as mybir
import concourse.tile as tile
from concourse import bacc
from concourse import bass_utils

C = 512          # channels
HW = 4096        # pixels per plane (64*64)
NQ = 2048        # query pixels per core
B = 4            # batch
N_CORES = 8
CT = C // 128    # channel tiles (4)
JT = HW // 128   # key tiles on partitions (32)
JP = JT // 2     # key tile pairs for DoubleRow (16)
IB = NQ // 512   # query i-blocks of 512 (4)
KNB = HW // 512  # key n-chunks for k projection (8)
EPS = 1e-5
SCALE = 1.0 / np.sqrt(np.float32(C))  # 1/sqrt(512)
EXP_OFF = -5.0   # exp offset; cancels in U/Z, keeps fp8 exp in range

F32 = mybir.dt.float32
BF16 = mybir.dt.bfloat16
FP8 = mybir.dt.float8e4


def build_nc():
    nc = bacc.Bacc("TRN2", target_bir_lowering=False, debug=False,
                   num_devices=N_CORES)
    x = nc.dram_tensor("x", [C, HW], BF16, kind="ExternalInput").ap()
    xq = nc.dram_tensor("xq", [C, NQ], F32, kind="ExternalInput").ap()
    # packed weights: w_rs[p, ct*C + o] = w.T[ct*128 + p, o]
    wqT = nc.dram_tensor("wqT", [128, CT * C], BF16, kind="ExternalInput").ap()
    wkT = nc.dram_tensor("wkT", [128, CT * C], BF16, kind="ExternalInput").ap()
    wvT = nc.dram_tensor("wvT", [128, CT * C], BF16, kind="ExternalInput").ap()
    wpT = nc.dram_tensor("wpT", [128, CT * C], BF16, kind="ExternalInput").ap()
    # packed biases: b_rs[p, ct] = b[ct*128 + p]
    bq = nc.dram_tensor("bq", [128, CT], F32, kind="ExternalInput").ap()
    bk = nc.dram_tensor("bk", [128, CT], F32, kind="ExternalInput").ap()
    out = nc.dram_tensor("out", [C, NQ], F32, kind="ExternalOutput").ap()

    with tile.TileContext(nc) as tc:
        build_graph(tc, x, xq, wqT, wkT, wvT, wpT, bq, bk, out)
    nc.compile()
    return nc


def build_graph(tc, x, xq, wqT, wkT, wvT, wpT, bq, bk, out):
    nc = tc.nc
    DR = mybir.MatmulPerfMode.DoubleRow
    AF = mybir.ActivationFunctionType
    with (
        tc.tile_pool(name="const", bufs=1) as const,
        tc.tile_pool(name="qk", bufs=1) as qkp,
        tc.tile_pool(name="vt", bufs=1) as vtp,
        tc.tile_pool(name="bc", bufs=1) as bcp,
    ):
        # ---- weights (one DMA each), biases, constants ----
        x_dma_insts = []
        w_dma_insts = []
        w_sb = {}
        for wname, wap in (("wq", wqT), ("wk", wkT), ("wv", wvT), ("wp", wpT)):
            t = const.tile([128, CT * C], BF16, tag=wname, name=wname)
            w_dma_insts.append(nc.sync.dma_start(out=t, in_=wap))
            w_sb[wname] = [t[:, ct * C:(ct + 1) * C] for ct in range(CT)]
        b_sb = {}
        for bname, bap in (("bq", bq), ("bk", bk)):
            t = const.tile([128, CT], F32, tag=bname, name=bname)
            nc.sync.dma_start(out=t, in_=bap)
            b_sb[bname] = [t[:, ct:ct + 1] for ct in range(CT)]
        ones_sb = const.tile([128, 128], F32, tag="ones", name="ones")
        nc.vector.memset(ones_sb, 1.0)
        eps_sb = const.tile([128, 1], F32, tag="eps", name="eps")
        nc.vector.memset(eps_sb, EPS)
        expoff_sb = const.tile([128, 1], F32, tag="expoff", name="expoff")
        nc.vector.memset(expoff_sb, EXP_OFF)

        # persistent activations (fp8 DoubleRow pair layouts)
        q_sb = [qkp.tile([128, 2, NQ], FP8, tag=f"q{g}", name=f"q{g}")
                for g in range(2)]
        k_sb = [qkp.tile([128, 2, HW], FP8, tag=f"k{g}", name=f"k{g}")
                for g in range(2)]
        vT_sb = [vtp.tile([128, 2, C], FP8, tag=f"vT{jtp}", name=f"vT{jtp}")
                 for jtp in range(JP)]

        # bias-correction result tiles (written by tiny matmuls below)
        qbias_sb = [bcp.tile([128, 1], F32, tag=f"qb{mt}", name=f"qb{mt}")
                    for mt in range(CT)]
        kbias_sb = [bcp.tile([128, 1], F32, tag=f"kb{mt}", name=f"kb{mt}")
                    for mt in range(CT)]
        cvnb_sb = [bcp.tile([128, 1], BF16, tag=f"cvn{mt}", name=f"cvn{mt}")
                   for mt in range(CT)]
        wpcv_sb = [bcp.tile([128, 1], F32, tag=f"wpcv{mt}", name=f"wpcv{mt}")
                   for mt in range(CT)]

        def q_epi_dst(ct2, nsl):
            return q_sb[ct2 // 2][:, ct2 % 2, nsl]

        def k_epi_dst(ct2, nsl):
            return k_sb[ct2 // 2][:, ct2 % 2, nsl]

        def vt_epi_dst(jt):
            return vT_sb[jt // 2][:, jt % 2, :]

        with (
            tc.tile_pool(name="xin", bufs=1) as xin,
            tc.tile_pool(name="stat", bufs=4) as stat,
            tc.tile_pool(name="ws", bufs=1) as wsp,
            tc.tile_pool(name="psB", bufs=6, space="PSUM") as psB,
        ):
            # ---- stage A: InstanceNorm stats -> rstd/nmb; scale weights ----
            x_sb = []
            rstd_sb, nmbb_sb = [], []
            scratch = stat.tile([128, HW], BF16, tag="scratch", name="scratch",
                                bufs=1)
            for ct in range(CT):
                x_t = xin.tile([128, HW], BF16, tag=f"xt{ct}", name=f"xt{ct}")
                for half in range(2):
                    x_dma_insts.append(nc.sync.dma_start(
                        out=x_t[:, half * 2048:(half + 1) * 2048],
                        in_=x[ct * 128:(ct + 1) * 128,
                              half * 2048:(half + 1) * 2048]))
                x_sb.append(x_t)
                mv = stat.tile([128, 2], F32, tag=f"mv{ct}", name=f"mv{ct}",
                               bufs=1)
                mu = mv[:, 0:1]
                var = mv[:, 1:2]
                if ct < 2:
                    # DVE path: bn_stats/bn_aggr
                    stats = stat.tile([128, 8, 6], F32, tag="stats",
                                      name=f"stats{ct}", bufs=2)
                    for sg in range(8):
                        nc.vector.bn_stats(out=stats[:, sg, :],
                                           in_=x_t[:, sg * 512:(sg + 1) * 512])
                    nc.vector.bn_aggr(out=mv, in_=stats)
                else:
                    # ACT path: sum(x) and sum(x^2) via accum_out
                    sx = stat.tile([128, 1], F32, tag=f"sx{ct}",
                                   name=f"sx{ct}", bufs=1)
                    sx2 = stat.tile([128, 1], F32, tag=f"sx2{ct}",
                                    name=f"sx2{ct}", bufs=1)
                    nc.scalar.activation(out=scratch, in_=x_t, func=AF.Copy,
                                         accum_out=sx)
                    nc.scalar.activation(out=scratch, in_=x_t, func=AF.Square,
                                         accum_out=sx2)
                    nc.vector.tensor_scalar_mul(mu, sx, 1.0 / HW)
                    # var = sum(x^2)/HW - mu^2
                    mu2 = stat.tile([128, 1], F32, tag=f"mu2{ct}",
                                    name=f"mu2{ct}", bufs=1)
                    nc.vector.tensor_mul(mu2, mu, mu)
                    nc.vector.tensor_scalar_mul(var, sx2, 1.0 / HW)
                    nc.vector.tensor_sub(var, var, mu2)
                # rstd = 1/sqrt(var + eps)
                std = stat.tile([128, 1], F32, tag=f"std{ct}",
                                name=f"std{ct}", bufs=1)
                nc.scalar.activation(out=std, in_=var, func=AF.Sqrt,
                                     bias=eps_sb, scale=1.0)
                rstd = stat.tile([128, 1], F32, tag=f"rstd{ct}",
                                 name=f"rstd{ct}", bufs=1)
                nc.vector.reciprocal(out=rstd, in_=std)
                rstd_sb.append(rstd)
                # nmb = -mu * rstd (bf16 copy feeds the bias matmuls)
                nmb = stat.tile([128, 1], F32, tag=f"nmb{ct}", name=f"nmb{ct}",
                                bufs=1)
                nc.vector.tensor_scalar_mul(nmb, mu, -1.0)
                nc.vector.tensor_mul(nmb, nmb, rstd)
                nmbb = stat.tile([128, 1], BF16, tag=f"nmbb{ct}",
                                 name=f"nmbb{ct}", bufs=1)
                nc.vector.tensor_copy(nmbb, nmb)
                nmbb_sb.append(nmbb)

            # keep weights off the DMA queues until x has landed - they
            # otherwise steal HBM bandwidth from the startup-critical load
            for wi in w_dma_insts:
                bass._add_dep_helper(wi.ins, x_dma_insts[-1].ins, sync=True,
                                     reason="x load first")

            # normalized-weight tiles: ws = wT * rstd (per input channel)
            ws = {}
            for wname in ("wq", "wk", "wv"):
                tiles = []
                for ct in range(CT):
                    t = wsp.tile([128, C], BF16, tag=f"{wname}s{ct}",
                                 name=f"{wname}s{ct}")
                    nc.vector.tensor_scalar_mul(t, w_sb[wname][ct],
                                                rstd_sb[ct])
                    tiles.append(t)
                ws[wname] = tiles

            # ---- stage B: projections on raw x with scaled weights ----
            # Main matmul groups are emitted first, the tiny bias matmuls
            # next, and the PSUM->SBUF epilogues last: the epilogues read the
            # bias tiles, so their writers must be emitted before them, but
            # the bias matmuls depend on stats from every channel tile and
            # would stall the PE if they preceded the main groups.
            def bias_mms(wname, badd, dsts):
                for mt in range(CT):
                    psb = psB.tile([128, 1], F32, tag="psBb", bufs=1,
                                   name=f"ps_{wname}b{mt}")
                    for ct in range(CT):
                        nc.tensor.matmul(
                            psb, w_sb[wname][ct][:, mt * 128:(mt + 1) * 128],
                            nmbb_sb[ct], start=(ct == 0), stop=(ct == CT - 1))
                    if badd is not None:
                        nc.scalar.activation(out=dsts[mt], in_=psb,
                                             func=AF.Identity,
                                             bias=badd[mt], scale=1.0)
                    else:
                        nc.scalar.activation(out=dsts[mt], in_=psb,
                                             func=AF.Copy)

            # q[ct2][:, n*512...] (only first NQ pixels)
            q_ps = {}
            for ct2 in range(CT):
                for n in range(IB):
                    nsl = slice(n * 512, (n + 1) * 512)
                    ps = psB.tile([128, 512], F32, tag="psB",
                                  name=f"psq{ct2}_{n}")
                    for ct in range(CT):
                        nc.tensor.matmul(
                            ps, ws["wq"][ct][:, ct2 * 128:(ct2 + 1) * 128],
                            x_sb[ct][:, nsl],
                            start=(ct == 0), stop=(ct == CT - 1))
                    q_ps[(ct2, n)] = ps
            bias_mms("wq", b_sb["bq"], qbias_sb)
            for (ct2, n), ps in q_ps.items():
                nsl = slice(n * 512, (n + 1) * 512)
                nc.scalar.activation(
                    out=q_epi_dst(ct2, nsl), in_=ps, func=AF.Identity,
                    bias=qbias_sb[ct2], scale=1.0)
            # k[ct2] over all HW pixels
            k_ps = {}
            for ct2 in range(CT):
                for n in range(KNB):
                    nsl = slice(n * 512, (n + 1) * 512)
                    ps = psB.tile([128, 512], F32, tag="psB",
                                  name=f"psk{ct2}_{n}")
                    for ct in range(CT):
                        nc.tensor.matmul(
                            ps, ws["wk"][ct][:, ct2 * 128:(ct2 + 1) * 128],
                            x_sb[ct][:, nsl],
                            start=(ct == 0), stop=(ct == CT - 1))
                    k_ps[(ct2, n)] = ps
            bias_mms("wk", b_sb["bk"], kbias_sb)
            for (ct2, n), ps in k_ps.items():
                nsl = slice(n * 512, (n + 1) * 512)
                nc.scalar.activation(
                    out=k_epi_dst(ct2, nsl), in_=ps, func=AF.Identity,
                    bias=kbias_sb[ct2], scale=1.0)
            # vT[jt] = [j=128, c=512]; v bias/const handled downstream
            for jt in range(JT):
                ps = psB.tile([128, 512], F32, tag="psB", name=f"psv{jt}")
                for ct in range(CT):
                    nc.tensor.matmul(
                        ps, x_sb[ct][:, jt * 128:(jt + 1) * 128],
                        ws["wv"][ct],
                        start=(ct == 0), stop=(ct == CT - 1))
                nc.vector.tensor_copy(vt_epi_dst(jt), ps)
            # cvn = wv @ nmb = -const_v; wpcv = wp @ cvn = -wp @ const_v,
            # added in the output epilogue
            bias_mms("wv", None, cvnb_sb)
            for mt in range(CT):
                psb = psB.tile([128, 1], F32, tag="psBb", bufs=1,
                               name=f"pswpcv{mt}")
                for ct in range(CT):
                    nc.tensor.matmul(
                        psb, w_sb["wp"][ct][:, mt * 128:(mt + 1) * 128],
                        cvnb_sb[ct], start=(ct == 0), stop=(ct == CT - 1))
                nc.scalar.activation(out=wpcv_sb[mt], in_=psb, func=AF.Copy)

        # ---- stage C: attention + output projection, per i-block ----
        with (
            tc.tile_pool(name="xres", bufs=2) as xresp,
            tc.tile_pool(name="expp", bufs=3) as expp,
            tc.tile_pool(name="op", bufs=2) as op,
            tc.tile_pool(name="yp", bufs=3) as yp,
            tc.tile_pool(name="rzp", bufs=2) as rzp,
            tc.tile_pool(name="zaccp", bufs=2) as zaccp,
            tc.tile_pool(name="psL", bufs=3, space="PSUM") as psLp,
            tc.tile_pool(name="psAcc", bufs=1, space="PSUM") as psAccp,
            tc.tile_pool(name="psP", bufs=1, space="PSUM") as psPp,
        ):
            for ib in range(IB):
                isl = slice(ib * 512, (ib + 1) * 512)
                psU = [psAccp.tile([128, 512], F32, tag=f"psU{ct}",
                                   name=f"psU{ct}_{ib}") for ct in range(CT)]
                # partial softmax denominator, accumulated on DVE
                zacc = zaccp.tile([128, 512], F32, tag="zacc", name=f"zacc{ib}")

                # software-pipelined j-loop: QK(jt+1) issues before U(jt)
                psL_tiles = [None] * JT
                exp_pair = [None]

                def emit_qk(jt):
                    ps = psLp.tile([128, 512], F32, tag="psL",
                                   name=f"psL{jt}_{ib}")
                    for g in range(2):
                        nc.tensor.matmul(
                            ps, k_sb[g][:, :, jt * 128:(jt + 1) * 128],
                            q_sb[g][:, :, isl],
                            start=(g == 0), stop=(g == 1), perf_mode=DR)
                    psL_tiles[jt] = ps

                emit_qk(0)
                first_exp_inst = None
                for jt in range(JT):
                    if jt % 2 == 0:
                        exp_pair[0] = expp.tile([128, 2, 512], FP8, tag="expT",
                                                name=f"expT{jt//2}_{ib}")
                    exp_dst = exp_pair[0][:, jt % 2, :]
                    einst = nc.scalar.activation(
                        out=exp_dst, in_=psL_tiles[jt], func=AF.Exp,
                        bias=expoff_sb, scale=float(SCALE))
                    if first_exp_inst is None:
                        first_exp_inst = einst
                    if jt + 1 < JT:
                        emit_qk(jt + 1)
                    if jt == 0:
                        nc.vector.tensor_copy(zacc, exp_dst)
                    else:
                        nc.vector.tensor_add(zacc, zacc, exp_dst)
                    if jt % 2 == 1:
                        jtp = jt // 2
                        for ct in range(CT):
                            nc.tensor.matmul(
                                psU[ct],
                                vT_sb[jtp][:, :, ct * 128:(ct + 1) * 128],
                                exp_pair[0],
                                start=(jtp == 0), stop=(jtp == JP - 1),
                                perf_mode=DR)

                # U copies split ACT/DVE - they free the psU banks for the
                # next i-block and feed the projection; emitted BEFORE the
                # reciprocal so the DVE ones don't queue behind it
                o_sb = []
                for ct in range(CT):
                    o_t = op.tile([128, 512], BF16, tag=f"o{ct}",
                                  name=f"o{ct}_{ib}")
                    if ct % 2 == 0:
                        nc.scalar.activation(out=o_t, in_=psU[ct],
                                             func=AF.Copy)
                    else:
                        nc.vector.tensor_copy(o_t, psU[ct])
                    o_sb.append(o_t)

                # partition-reduce + broadcast the denominator in one f32
                # matmul: psZb[p, i] = sum_j zacc[j, i] for every p.
                # 1/Z is applied at the output epilogue, off the PE path.
                # psZb shares the psP bank (same tag) - free before the first
                # projection matmul needs it.
                psZb = psPp.tile([128, 512], F32, tag="psP", name=f"psZb{ib}")
                nc.tensor.matmul(psZb, ones_sb, zacc, start=True, stop=True)
                rzb = rzp.tile([128, 512], F32, tag="rzb", name=f"rzb{ib}")
                nc.vector.reciprocal_approx_fast(out=rzb, in_=psZb)

                # output projection (on unnormalized U), then
                # y = psP/Z + (-wp@const_v) + (xq + wp@bv + bp)
                for mt in range(CT):
                    psP = psPp.tile([128, 512], F32, tag="psP",
                                    name=f"psP{mt}_{ib}")
                    for ct in range(CT):
                        nc.tensor.matmul(
                            psP, w_sb["wp"][ct][:, mt * 128:(mt + 1) * 128],
                            o_sb[ct], start=(ct == 0), stop=(ct == CT - 1))
                    y = yp.tile([128, 512], F32, tag="y", name=f"y{mt}_{ib}")
                    nc.vector.tensor_mul(y, psP, rzb)
                    nc.vector.tensor_scalar_add(y, y, wpcv_sb[mt])
                    xr = xresp.tile([128, 512], F32, tag="xr",
                                    name=f"xr{mt}_{ib}")
                    xr_dma = nc.sync.dma_start(
                        out=xr, in_=xq[mt * 128:(mt + 1) * 128, isl])
                    # keep the residual loads off the DMA queues until this
                    # i-block's attention is underway - they'd otherwise
                    # compete with the startup x load for HBM bandwidth
                    bass._add_dep_helper(xr_dma.ins, first_exp_inst.ins,
                                         sync=True,
                                         reason="delay residual load")
                    nc.vector.tensor_add(y, y, xr)
                    nc.sync.dma_start(out=out[mt * 128:(mt + 1) * 128, isl],
                                      in_=y)


_NC = None


def _get_nc():
    global _NC
    if _NC is None:
        _NC = build_nc()
    return _NC


def make_in_maps(x, wq, bq, wk, bk, wv, bv, wp, bp):
    x = np.asarray(x, dtype=np.float32)
    wq, wk, wv, wp = (np.asarray(a, dtype=np.float32) for a in (wq, wk, wv, wp))
    bq, bk, bv, bp = (np.asarray(a, dtype=np.float32) for a in (bq, bk, bv, bp))
    bp2 = wp @ bv + bp

    def pack_w(w):
        # [p, ct*C + o] = w.T[ct*128 + p, o]
        wT = np.ascontiguousarray(w.T)
        return np.ascontiguousarray(
            wT.reshape(CT, 128, C).transpose(1, 0, 2).reshape(128, CT * C)
        ).astype(ml_dtypes.bfloat16)

    def pack_b(b):
        return np.ascontiguousarray(b.reshape(CT, 128).T).astype(np.float32)

    shared = {
        "wqT": pack_w(wq), "wkT": pack_w(wk), "wvT": pack_w(wv),
        "wpT": pack_w(wp),
        "bq": pack_b(bq), "bk": pack_b(bk),
    }
    in_maps = []
    for core in range(N_CORES):
        b, h = divmod(core, 2)
        xb = x[b].reshape(C, HW)
        xc = np.roll(xb, -h * NQ, axis=1)  # queries at columns [0, NQ)
        in_maps.append({
            "x": np.ascontiguousarray(xc).astype(ml_dtypes.bfloat16),
            "xq": np.ascontiguousarray(xc[:, :NQ]) + bp2[:, None],
            **shared,
        })
    return in_maps


def assemble_out(results):
    out = np.empty((B, C, HW), dtype=np.float32)
    for core in range(N_CORES):
        b, h = divmod(core, 2)
        out[b][:, h * NQ:(h + 1) * NQ] = results[core]["out"]
    return out.reshape(B, C, 64, 64)


def kernel(x, wq, bq, wk, bk, wv, bv, wp, bp):
    nc = _get_nc()
    in_maps = make_in_maps(x, wq, bq, wk, bk, wv, bv, wp, bp)
    res = bass_utils.run_bass_kernel_spmd(nc, in_maps,
                                          core_ids=list(range(N_CORES)))
    return assemble_out(res.results)


# revision 25
# speedup vs baseline: 1.0103x; 1.0103x over previous
"""AttentionBlock (InstanceNorm + single-head self-attention over 64x64 pixels
+ residual) on 8 Trainium2 NeuronCores.

Sharding: core = (batch b = core//2, query-half h = core%2). Each core gets the
full 512x4096 plane of its batch (columns rolled so its 2048 query pixels are
columns 0..2047), computes norm + K/V for all 4096 pixels and Q for its 2048,
runs softmax(Q^T K / sqrt(C)) V and the output projection for its half, and
returns a [512, 2048] shard. No collectives; the graph is SPMD-identical and
per-core differences enter only through the input data.

The InstanceNorm is folded into the projections: q/k/v = W @ ((x-mu)*rstd) + b
= (W*rstd) @ x + (b + W @ nmb) with nmb = -mu*rstd, so the projection matmuls
run directly on raw x as soon as the per-channel stats are known - the
normalized tensor is never materialized. The bias corrections W @ nmb are
[C,1] matmuls (cheap).

The attention is computed in the transposed orientation,
logitsT[j, i] = <k_j, q_i>, so the softmax denominator sum_j exp() is a
ones-matmul over the partition axis and exp() needs no max-subtraction
(logits are bounded for this data distribution; exp carries a -5 offset so
fp8 storage cannot overflow - the offset cancels exactly in U/Z).
The v bias passes through softmax exactly (attention rows sum to 1), so it is
folded into the residual on the host: xq' = xq + wp @ bv + bp. The v "const"
term wv @ nmb also passes through softmax; it is projected through wp on-chip
([C,1] matmuls) and added in the output epilogue.

QK^T and exp()V run as fp8(e4m3) DoubleRow matmuls - 256-deep contraction per
instruction, half the PE instructions of bf16 - with fp32 PSUM accumulation.
Everything else is bf16 with fp32 accumulation.
"""

import numpy as np
import ml_dtypes

import concourse.bass as bass
import concourse.mybir as mybir
import concourse.tile as tile
from concourse import bacc
from concourse import bass_utils

C = 512          # channels
HW = 4096        # pixels per plane (64*64)
NQ = 2048        # query pixels per core
B = 4            # batch
N_CORES = 8
CT = C // 128    # channel tiles (4)
JT = HW // 128   # key tiles on partitions (32)
JP = JT // 2     # key tile pairs for DoubleRow (16)
IB = NQ // 512   # query i-blocks of 512 (4)
KNB = HW // 512  # key n-chunks for k projection (8)
EPS = 1e-5
SCALE = 1.0 / np.sqrt(np.float32(C))  # 1/sqrt(512)
EXP_OFF = -5.0   # exp offset; cancels in U/Z, keeps fp8 exp in range

F32 = mybir.dt.float32
BF16 = mybir.dt.bfloat16
FP8 = mybir.dt.float8e4


def build_nc():
    nc = bacc.Bacc("TRN2", target_bir_lowering=False, debug=False,
                   num_devices=N_CORES)
    x = nc.dram_tensor("x", [C, HW], BF16, kind="ExternalInput").ap()
    xq = nc.dram_tensor("xq", [C, NQ], F32, kind="ExternalInput").ap()
    # packed weights: w_rs[p, ct*C + o] = w.T[ct*128 + p, o]
    wqT = nc.dram_tensor("wqT", [128, CT * C], BF16, kind="ExternalInput").ap()
    wkT = nc.dram_tensor("wkT", [128, CT * C], BF16, kind="ExternalInput").ap()
    wvT = nc.dram_tensor("wvT", [128, CT * C], BF16, kind="ExternalInput").ap()
    wpT = nc.dram_tensor("wpT", [128, CT * C], BF16, kind="ExternalInput").ap()
    # packed biases: b_rs[p, ct] = b[ct*128 + p]
    bq = nc.dram_tensor("bq", [128, CT], F32, kind="ExternalInput").ap()
    bk = nc.dram_tensor("bk", [128, CT], F32, kind="ExternalInput").ap()
    out = nc.dram_tensor("out", [C, NQ], F32, kind="ExternalOutput").ap()

    with tile.TileContext(nc) as tc:
        build_graph(tc, x, xq, wqT, wkT, wvT, wpT, bq, bk, out)
    nc.compile()
    return nc


def build_graph(tc, x, xq, wqT, wkT, wvT, wpT, bq, bk, out):
    nc = tc.nc
    DR = mybir.MatmulPerfMode.DoubleRow
    AF = mybir.ActivationFunctionType
    with (
        tc.tile_pool(name="const", bufs=1) as const,
        tc.tile_pool(name="qk", bufs=1) as qkp,
        tc.tile_pool(name="vt", bufs=1) as vtp,
        tc.tile_pool(name="bc", bufs=1) as bcp,
    ):
        # ---- weights (one DMA each), biases, constants ----
        x_dma_insts = []
        w_dma_insts = []
        w_sb = {}
        for wname, wap in (("wq", wqT), ("wk", wkT), ("wv", wvT), ("wp", wpT)):
            t = const.tile([128, CT * C], BF16, tag=wname, name=wname)
            w_dma_insts.append(nc.sync.dma_start(out=t, in_=wap))
            w_sb[wname] = [t[:, ct * C:(ct + 1) * C] for ct in range(CT)]
        b_sb = {}
        for bname, bap in (("bq", bq), ("bk", bk)):
            t = const.tile([128, CT], F32, tag=bname, name=bname)
            nc.sync.dma_start(out=t, in_=bap)
            b_sb[bname] = [t[:, ct:ct + 1] for ct in range(CT)]
        ones_sb = const.tile([128, 128], F32, tag="ones", name="ones")
        nc.vector.memset(ones_sb, 1.0)
        eps_sb = const.tile([128, 1], F32, tag="eps", name="eps")
        nc.vector.memset(eps_sb, EPS)
        expoff_sb = const.tile([128, 1], F32, tag="expoff", name="expoff")
        nc.vector.memset(expoff_sb, EXP_OFF)

        # persistent activations (fp8 DoubleRow pair layouts)
        q_sb = [qkp.tile([128, 2, NQ], FP8, tag=f"q{g}", name=f"q{g}")
                for g in range(2)]
        k_sb = [qkp.tile([128, 2, HW], FP8, tag=f"k{g}", name=f"k{g}")
                for g in range(2)]
        vT_sb = [vtp.tile([128, 2, C], FP8, tag=f"vT{jtp}", name=f"vT{jtp}")
                 for jtp in range(JP)]

        # bias-correction result tiles (written by tiny matmuls below)
        qbias_sb = [bcp.tile([128, 1], F32, tag=f"qb{mt}", name=f"qb{mt}")
                    for mt in range(CT)]
        kbias_sb = [bcp.tile([128, 1], F32, tag=f"kb{mt}", name=f"kb{mt}")
                    for mt in range(CT)]
        cvnb_sb = [bcp.tile([128, 1], BF16, tag=f"cvn{mt}", name=f"cvn{mt}")
                   for mt in range(CT)]
        wpcv_sb = [bcp.tile([128, 1], F32, tag=f"wpcv{mt}", name=f"wpcv{mt}")
                   for mt in range(CT)]

        def q_epi_dst(ct2, nsl):
            return q_sb[ct2 // 2][:, ct2 % 2, nsl]

        def k_epi_dst(ct2, nsl):
            return k_sb[ct2 // 2][:, ct2 % 2, nsl]

        def vt_epi_dst(jt):
            return vT_sb[jt // 2][:, jt % 2, :]

        with (
            tc.tile_pool(name="xin", bufs=1) as xin,
            tc.tile_pool(name="stat", bufs=4) as stat,
            tc.tile_pool(name="ws", bufs=1) as wsp,
            tc.tile_pool(name="psB", bufs=6, space="PSUM") as psB,
        ):
            # ---- stage A: InstanceNorm stats -> rstd/nmb; scale weights ----
            x_sb = []
            rstd_sb, nmbb_sb = [], []
            scratch = stat.tile([128, HW], BF16, tag="scratch", name="scratch",
                                bufs=1)
            for ct in range(CT):
                x_t = xin.tile([128, HW], BF16, tag=f"xt{ct}", name=f"xt{ct}")
                for half in range(2):
                    x_dma_insts.append(nc.sync.dma_start(
                        out=x_t[:, half * 2048:(half + 1) * 2048],
                        in_=x[ct * 128:(ct + 1) * 128,
                              half * 2048:(half + 1) * 2048]))
                x_sb.append(x_t)
                mv = stat.tile([128, 2], F32, tag=f"mv{ct}", name=f"mv{ct}",
                               bufs=1)
                mu = mv[:, 0:1]
                var = mv[:, 1:2]
                if ct < 2:
                    # DVE path: bn_stats/bn_aggr
                    stats = stat.tile([128, 8, 6], F32, tag="stats",
                                      name=f"stats{ct}", bufs=2)
                    for sg in range(8):
                        nc.vector.bn_stats(out=stats[:, sg, :],
                                           in_=x_t[:, sg * 512:(sg + 1) * 512])
                    nc.vector.bn_aggr(out=mv, in_=stats)
                else:
                    # ACT path: sum(x) and sum(x^2) via accum_out
                    sx = stat.tile([128, 1], F32, tag=f"sx{ct}",
                                   name=f"sx{ct}", bufs=1)
                    sx2 = stat.tile([128, 1], F32, tag=f"sx2{ct}",
                                    name=f"sx2{ct}", bufs=1)
                    nc.scalar.activation(out=scratch, in_=x_t, func=AF.Copy,
                                         accum_out=sx)
                    nc.scalar.activation(out=scratch, in_=x_t, func=AF.Square,
                                         accum_out=sx2)
                    nc.vector.tensor_scalar_mul(mu, sx, 1.0 / HW)
                    # var = sum(x^2)/HW - mu^2
                    mu2 = stat.tile([128, 1], F32, tag=f"mu2{ct}",
                                    name=f"mu2{ct}", bufs=1)
                    nc.vector.tensor_mul(mu2, mu, mu)
                    nc.vector.tensor_scalar_mul(var, sx2, 1.0 / HW)
                    nc.vector.tensor_sub(var, var, mu2)
                # rstd = 1/sqrt(var + eps)
                std = stat.tile([128, 1], F32, tag=f"std{ct}",
                                name=f"std{ct}", bufs=1)
                nc.scalar.activation(out=std, in_=var, func=AF.Sqrt,
                                     bias=eps_sb, scale=1.0)
                rstd = stat.tile([128, 1], F32, tag=f"rstd{ct}",
                                 name=f"rstd{ct}", bufs=1)
                nc.vector.reciprocal(out=rstd, in_=std)
                rstd_sb.append(rstd)
                # nmb = -mu * rstd (bf16 copy feeds the bias matmuls)
                nmb = stat.tile([128, 1], F32, tag=f"nmb{ct}", name=f"nmb{ct}",
                                bufs=1)
                nc.vector.tensor_scalar_mul(nmb, mu, -1.0)
                nc.vector.tensor_mul(nmb, nmb, rstd)
                nmbb = stat.tile([128, 1], BF16, tag=f"nmbb{ct}",
                                 name=f"nmbb{ct}", bufs=1)
                nc.vector.tensor_copy(nmbb, nmb)
                nmbb_sb.append(nmbb)

            # keep weights off the DMA queues until x has landed - they
            # otherwise steal HBM bandwidth from the startup-critical load
            for wi in w_dma_insts:
                bass._add_dep_helper(wi.ins, x_dma_insts[-1].ins, sync=True,
                                     reason="x load first")

            # normalized-weight tiles: ws = wT * rstd (per input channel)
            ws = {}
            for wname in ("wq", "wk", "wv"):
                tiles = []
                for ct in range(CT):
                    t = wsp.tile([128, C], BF16, tag=f"{wname}s{ct}",
                                 name=f"{wname}s{ct}")
                    nc.vector.tensor_scalar_mul(t, w_sb[wname][ct],
                                                rstd_sb[ct])
                    tiles.append(t)
                ws[wname] = tiles

            # ---- stage B: projections on raw x with scaled weights ----
            # Main matmul groups are emitted first, the tiny bias matmuls
            # next, and the PSUM->SBUF epilogues last: the epilogues read the
            # bias tiles, so their writers must be emitted before them, but
            # the bias matmuls depend on stats from every channel tile and
            # would stall the PE if they preceded the main groups.
            def bias_mms(wname, badd, dsts):
                for mt in range(CT):
                    psb = psB.tile([128, 1], F32, tag="psBb", bufs=1,
                                   name=f"ps_{wname}b{mt}")
                    for ct in range(CT):
                        nc.tensor.matmul(
                            psb, w_sb[wname][ct][:, mt * 128:(mt + 1) * 128],
                            nmbb_sb[ct], start=(ct == 0), stop=(ct == CT - 1))
                    if badd is not None:
                        nc.scalar.activation(out=dsts[mt], in_=psb,
                                             func=AF.Identity,
                                             bias=badd[mt], scale=1.0)
                    else:
                        nc.scalar.activation(out=dsts[mt], in_=psb,
                                             func=AF.Copy)

            # q[ct2][:, n*512...] (only first NQ pixels)
            q_ps = {}
            for ct2 in range(CT):
                for n in range(IB):
                    nsl = slice(n * 512, (n + 1) * 512)
                    ps = psB.tile([128, 512], F32, tag="psB",
                                  name=f"psq{ct2}_{n}")
                    for ct in range(CT):
                        nc.tensor.matmul(
                            ps, ws["wq"][ct][:, ct2 * 128:(ct2 + 1) * 128],
                            x_sb[ct][:, nsl],
                            start=(ct == 0), stop=(ct == CT - 1))
                    q_ps[(ct2, n)] = ps
            bias_mms("wq", b_sb["bq"], qbias_sb)
            for (ct2, n), ps in q_ps.items():
                nsl = slice(n * 512, (n + 1) * 512)
                nc.scalar.activation(
                    out=q_epi_dst(ct2, nsl), in_=ps, func=AF.Identity,
                    bias=qbias_sb[ct2], scale=1.0)
            # k[ct2] over all HW pixels
            k_ps = {}
            for ct2 in range(CT):
                for n in range(KNB):
                    nsl = slice(n * 512, (n + 1) * 512)
                    ps = psB.tile([128, 512], F32, tag="psB",
                                  name=f"psk{ct2}_{n}")
                    for ct in range(CT):
                        nc.tensor.matmul(
                            ps, ws["wk"][ct][:, ct2 * 128:(ct2 + 1) * 128],
                            x_sb[ct][:, nsl],
                            start=(ct == 0), stop=(ct == CT - 1))
                    k_ps[(ct2, n)] = ps
            bias_mms("wk", b_sb["bk"], kbias_sb)
            for (ct2, n), ps in k_ps.items():
                nsl = slice(n * 512, (n + 1) * 512)
                if n % 2 == 0:
                    nc.scalar.activation(
                        out=k_epi_dst(ct2, nsl), in_=ps, func=AF.Identity,
                        bias=kbias_sb[ct2], scale=1.0)
                else:
                    nc.vector.tensor_scalar_add(k_epi_dst(ct2, nsl), ps,
                                                kbias_sb[ct2])
            # vT[jt] = [j=128, c=512]; v bias/const handled downstream
            for jt in range(JT):
                ps = psB.tile([128, 512], F32, tag="psB", name=f"psv{jt}")
                for ct in range(CT):
                    nc.tensor.matmul(
                        ps, x_sb[ct][:, jt * 128:(jt + 1) * 128],
                        ws["wv"][ct],
                        start=(ct == 0), stop=(ct == CT - 1))
                nc.vector.tensor_copy(vt_epi_dst(jt), ps)
            # cvn = wv @ nmb = -const_v; wpcv = wp @ cvn = -wp @ const_v,
            # added in the output epilogue
            bias_mms("wv", None, cvnb_sb)
            for mt in range(CT):
                psb = psB.tile([128, 1], F32, tag="psBb", bufs=1,
                               name=f"pswpcv{mt}")
                for ct in range(CT):
                    nc.tensor.matmul(
                        psb, w_sb["wp"][ct][:, mt * 128:(mt + 1) * 128],
                        cvnb_sb[ct], start=(ct == 0), stop=(ct == CT - 1))
                nc.scalar.activation(out=wpcv_sb[mt], in_=psb, func=AF.Copy)

        # ---- stage C: attention + output projection, per i-block ----
        with (
            tc.tile_pool(name="xres", bufs=2) as xresp,
            tc.tile_pool(name="expp", bufs=3) as expp,
            tc.tile_pool(name="op", bufs=2) as op,
            tc.tile_pool(name="yp", bufs=3) as yp,
            tc.tile_pool(name="rzp", bufs=2) as rzp,
            tc.tile_pool(name="zaccp", bufs=2) as zaccp,
            tc.tile_pool(name="psL", bufs=3, space="PSUM") as psLp,
            tc.tile_pool(name="psAcc", bufs=1, space="PSUM") as psAccp,
            tc.tile_pool(name="psP", bufs=1, space="PSUM") as psPp,
        ):
            for ib in range(IB):
                isl = slice(ib * 512, (ib + 1) * 512)
                psU = [psAccp.tile([128, 512], F32, tag=f"psU{ct}",
                                   name=f"psU{ct}_{ib}") for ct in range(CT)]
                # partial softmax denominator, accumulated on DVE
                zacc = zaccp.tile([128, 512], F32, tag="zacc", name=f"zacc{ib}")

                # software-pipelined j-loop: QK(jt+1) issues before U(jt)
                psL_tiles = [None] * JT
                exp_pair = [None]

                def emit_qk(jt):
                    ps = psLp.tile([128, 512], F32, tag="psL",
                                   name=f"psL{jt}_{ib}")
                    for g in range(2):
                        nc.tensor.matmul(
                            ps, k_sb[g][:, :, jt * 128:(jt + 1) * 128],
                            q_sb[g][:, :, isl],
                            start=(g == 0), stop=(g == 1), perf_mode=DR)
                    psL_tiles[jt] = ps

                emit_qk(0)
                first_exp_inst = None
                for jt in range(JT):
                    if jt % 2 == 0:
                        exp_pair[0] = expp.tile([128, 2, 512], FP8, tag="expT",
                                                name=f"expT{jt//2}_{ib}")
                    exp_dst = exp_pair[0][:, jt % 2, :]
                    einst = nc.scalar.activation(
                        out=exp_dst, in_=psL_tiles[jt], func=AF.Exp,
                        bias=expoff_sb, scale=float(SCALE))
                    if first_exp_inst is None:
                        first_exp_inst = einst
                    if jt + 1 < JT:
                        emit_qk(jt + 1)
                    if jt == 0:
                        nc.vector.tensor_copy(zacc, exp_dst)
                    else:
                        nc.vector.tensor_add(zacc, zacc, exp_dst)
                    if jt % 2 == 1:
                        jtp = jt // 2
                        for ct in range(CT):
                            nc.tensor.matmul(
                                psU[ct],
                                vT_sb[jtp][:, :, ct * 128:(ct + 1) * 128],
                                exp_pair[0],
                                start=(jtp == 0), stop=(jtp == JP - 1),
                                perf_mode=DR)

                # U copies split ACT/DVE - they free the psU banks for the
                # next i-block and feed the projection; emitted BEFORE the
                # reciprocal so the DVE ones don't queue behind it
                o_sb = []
                for ct in range(CT):
                    o_t = op.tile([128, 512], BF16, tag=f"o{ct}",
                                  name=f"o{ct}_{ib}")
                    if ct % 2 == 0:
                        nc.scalar.activation(out=o_t, in_=psU[ct],
                                             func=AF.Copy)
                    else:
                        nc.vector.tensor_copy(o_t, psU[ct])
                    o_sb.append(o_t)

                # partition-reduce + broadcast the denominator in one f32
                # matmul: psZb[p, i] = sum_j zacc[j, i] for every p.
                # 1/Z is applied at the output epilogue, off the PE path.
                # psZb shares the psP bank (same tag) - free before the first
                # projection matmul needs it.
                psZb = psPp.tile([128, 512], F32, tag="psP", name=f"psZb{ib}")
                nc.tensor.matmul(psZb, ones_sb, zacc, start=True, stop=True)
                rzb = rzp.tile([128, 512], F32, tag="rzb", name=f"rzb{ib}")
                nc.vector.reciprocal_approx_fast(out=rzb, in_=psZb)

                # output projection (on unnormalized U), then
                # y = psP/Z + (-wp@const_v) + (xq + wp@bv + bp)
                for mt in range(CT):
                    if ib == IB - 1 and mt % 2 == 1:
                        psP = psLp.tile([128, 512], F32, tag="psL",
                                        name=f"psP{mt}_{ib}")
                    else:
                        psP = psPp.tile([128, 512], F32, tag="psP",
                                        name=f"psP{mt}_{ib}")
                    for ct in range(CT):
                        nc.tensor.matmul(
                            psP, w_sb["wp"][ct][:, mt * 128:(mt + 1) * 128],
                            o_sb[ct], start=(ct == 0), stop=(ct == CT - 1))
                    y = yp.tile([128, 512], F32, tag="y", name=f"y{mt}_{ib}")
                    nc.vector.tensor_mul(y, psP, rzb)
                    nc.vector.tensor_scalar_add(y, y, wpcv_sb[mt])
                    xr = xresp.tile([128, 512], F32, tag="xr",
                                    name=f"xr{mt}_{ib}")
                    xr_dma = nc.sync.dma_start(
                        out=xr, in_=xq[mt * 128:(mt + 1) * 128, isl])
                    # keep the residual loads off the DMA queues until this
                    # i-block's attention is underway - they'd otherwise
                    # compete with the startup x load for HBM bandwidth
                    bass._add_dep_helper(xr_dma.ins, first_exp_inst.ins,
                                         sync=True,
                                         reason="delay residual load")
                    nc.vector.tensor_add(y, y, xr)
                    nc.sync.dma_start(out=out[mt * 128:(mt + 1) * 128, isl],
                                      in_=y)


_NC = None


def _get_nc():
    global _NC
    if _NC is None:
        _NC = build_nc()
    return _NC


def make_in_maps(x, wq, bq, wk, bk, wv, bv, wp, bp):
    x = np.asarray(x, dtype=np.float32)
    wq, wk, wv, wp = (np.asarray(a, dtype=np.float32) for a in (wq, wk, wv, wp))
    bq, bk, bv, bp = (np.asarray(a, dtype=np.float32) for a in (bq, bk, bv, bp))
    bp2 = wp @ bv + bp

    def pack_w(w):
        # [p, ct*C + o] = w.T[ct*128 + p, o]
        wT = np.ascontiguousarray(w.T)
        return np.ascontiguousarray(
            wT.reshape(CT, 128, C).transpose(1, 0, 2).reshape(128, CT * C)
        ).astype(ml_dtypes.bfloat16)

    def pack_b(b):
        return np.ascontiguousarray(b.reshape(CT, 128).T).astype(np.float32)

    shared = {
        "wqT": pack_w(wq), "wkT": pack_w(wk), "wvT": pack_w(wv),
        "wpT": pack_w(wp),
        "bq": pack_b(bq), "bk": pack_b(bk),
    }
    in_maps = []
    for core in range(N_CORES):
        b, h = divmod(core, 2)
        xb = x[b].reshape(C, HW)
        xc = np.roll(xb, -h * NQ, axis=1)  # queries at columns [0, NQ)
        in_maps.append({
            "x": np.ascontiguousarray(xc).astype(ml_dtypes.bfloat16),
            "xq": np.ascontiguousarray(xc[:, :NQ]) + bp2[:, None],
            **shared,
        })
    return in_maps


def assemble_out(results):
    out = np.empty((B, C, HW), dtype=np.float32)
    for core in range(N_CORES):
        b, h = divmod(core, 2)
        out[b][:, h * NQ:(h + 1) * NQ] = results[core]["out"]
    return out.reshape(B, C, 64, 64)


def kernel(x, wq, bq, wk, bk, wv, bv, wp, bp):
    nc = _get_nc()
    in_maps = make_in_maps(x, wq, bq, wk, bk, wv, bv, wp, bp)
    res = bass_utils.run_bass_kernel_spmd(nc, in_maps,
                                          core_ids=list(range(N_CORES)))
    return assemble_out(res.results)
